# revision 10
# baseline (speedup 1.0000x reference)
"""HashGrid embedding lookup (nn_HashGridPyTorch) as a TRN2 Bass kernel.

Strategy
--------
Data-parallel over the point batch: 2^20 points are split across 8
NeuronCores (131072 points each). The 57MB concatenated hash tables are
replicated to every core's HBM and gathered from DRAM with indirect DMAs.

Per core, per tile of 128x256 points:
  1. DMA the [128, 256*3] point tile into SBUF.
  2. DVE computes I15_c = clip(floor(x_norm * 2^19), 0, 2^19-1) per coord
     (robust floor: works for truncating or rounding fp32->int casts).
     Because every level resolution is a power of two (16 << l), the level-l
     cell coords are just I15 >> (15-l).
  3. The spatial hash h_l = (c1*ic0 + c2*ic1 + c3*ic2) mod 2^19 is computed
     with a bit-recursion across levels:
         h_l = (2*h_{l-1} + sum_c c_c*bit_c) mod 2^19
     keeping every intermediate exactly representable in fp32 (DVE's ALU is
     fp32 for arithmetic; int32 is only used for shifts/masks).
  4. idx_l = min(h_l, size_l-1) + offset_l  (int32)
  5. 16 indirect DMA gathers (one per level) pull tables[idx] (8B rows) from
     DRAM straight into the interleaved [128, 256, 16*2] output tile.
  6. One contiguous DMA writes the tile to the output.
"""

import numpy as np

import concourse.bass as bass
import concourse.bacc as bacc
import concourse.tile as tile
from concourse import mybir
from concourse.bass_utils import run_bass_kernel_spmd

# ---------------------------------------------------------------- constants
L = 16
F = 2
LOG2 = 19
MASK = (1 << LOG2) - 1
C = (73856093, 19349663, 83492791)
CM = tuple(c % (1 << LOG2) for c in C)

RES = [16 << l for l in range(L)]
SIZES = [min(1 << LOG2, (r + 1) ** 3) for r in RES]
OFFSETS = np.concatenate([[0], np.cumsum(SIZES)[:-1]]).astype(np.int64)
TOTAL_PARAMS = int(np.sum(SIZES))  # 7131219

B = 1 << 20
N_CORES = 8
B_CORE = B // N_CORES  # 131072

P = 128          # SBUF partitions
T = 256          # points per partition per tile
NTILES = B_CORE // (P * T)  # 4

f32 = mybir.dt.float32
i32 = mybir.dt.int32


def _emit_tile_program(nc, tc, pools, x_dram, tables_ap, out_dram, ti, size_val):
    """Emit one tile's worth of instructions (128*T points)."""
    io, work, idxp, outp = pools
    Alu = mybir.AluOpType

    add_c = float(size_val)
    scale_c = float((1 << LOG2) / (2.0 * size_val))

    xin = io.tile([P, 3 * T], f32, tag="xin")
    nc.sync.dma_start(out=xin[:], in_=x_dram[ti])

    # view [P, T, 3] -> coord planes with stride 3
    xv = xin[:].rearrange("p (t c) -> p c t", c=3)

    i15 = []
    for c in range(3):
        X = work.tile([P, T], f32, tag=f"X{c}")
        # X = (x + size) * (2^19 / (2 size))   (== (x+1) * 2^18 for size=1)
        nc.vector.tensor_scalar(X[:], xv[:, c, :], add_c, scale_c, Alu.add, Alu.mult)
        Fi = work.tile([P, T], i32, tag=f"Fi{c}")
        nc.vector.tensor_copy(Fi[:], X[:])                # fp32 -> int32 cast
        Ff = work.tile([P, T], f32, tag=f"Ff{c}")
        nc.vector.tensor_copy(Ff[:], Fi[:])               # back to fp32
        gt = work.tile([P, T], f32, tag=f"gt{c}")
        nc.vector.tensor_tensor(gt[:], Ff[:], X[:], Alu.is_gt)
        nc.vector.tensor_sub(Ff[:], Ff[:], gt[:])         # robust floor
        nc.vector.tensor_scalar(Ff[:], Ff[:], 0.0, float(MASK), Alu.max, Alu.min)
        Ii = work.tile([P, T], i32, tag=f"I15{c}")
        nc.vector.tensor_copy(Ii[:], Ff[:])               # exact int
        i15.append(Ii)

    otile = outp.tile([P, T, L * F], f32, tag="otile")
    # interleaved (t, l) index tile so ONE indirect DMA covers all levels and
    # the gathered rows land exactly in the interleaved output layout
    idx_all = idxp.tile([P, T * L], i32, tag="idx_all")
    idx_av = idx_all[:].rearrange("p (t l) -> p l t", l=L)

    # All-int32 pipeline from here on: bitVec ops (shift/and) cannot cast
    # dtypes on TRN2, and arith ops on int32 go through the fp32 ALU, which
    # is exact for every intermediate here (all values < 2^23).

    # ---- level 0: ic0 = I15 >> 15 (4-bit coords), direct masked products
    h = work.tile([P, T], i32, tag="h")
    acc = work.tile([P, T], i32, tag="acc")
    for c in range(3):
        # s = I15 >> 15 ; prod = s * CM[c] (< 15*2^19 < 2^23) ; mask to < 2^19
        nc.vector.tensor_scalar(acc[:], i15[c][:], 15, None, Alu.logical_shift_right)
        nc.vector.tensor_scalar(acc[:], acc[:], CM[c], None, Alu.mult)
        if c == 0:
            nc.vector.tensor_scalar(h[:], acc[:], MASK, None, Alu.bitwise_and)
        else:
            nc.vector.tensor_scalar(acc[:], acc[:], MASK, None, Alu.bitwise_and)
            nc.vector.tensor_add(h[:], h[:], acc[:])
    nc.vector.tensor_scalar(h[:], h[:], MASK, None, Alu.bitwise_and)

    def emit_level(l):
        nc.vector.tensor_scalar(
            idx_av[:, l, :], h[:], int(SIZES[l] - 1), int(OFFSETS[l]), Alu.min, Alu.add
        )

    emit_level(0)

    # ---- levels 1..15: h = (2h + sum_c CM[c] * bit_c(k)) mod 2^19
    for l in range(1, L):
        k = 15 - l
        nc.vector.tensor_scalar(h[:], h[:], 2, None, Alu.mult)
        for c in range(3):
            bit = work.tile([P, T], i32, tag=f"bit{c}")
            nc.vector.tensor_scalar(
                bit[:], i15[c][:], k, 1, Alu.logical_shift_right, Alu.bitwise_and
            )
            nc.vector.tensor_scalar(acc[:], bit[:], CM[c], None, Alu.mult)
            nc.vector.tensor_add(h[:], h[:], acc[:])
        nc.vector.tensor_scalar(h[:], h[:], MASK, None, Alu.bitwise_and)
        emit_level(l)

    # ---- gathers: walrus only supports one dynamic offset per partition per
    # DMA (offset [P,1], 2D out [P,D] -> D contiguous elems from tables[idx]).
    # One DMA per (t, l) slot: 128 lookups each.
    oflat = otile[:].rearrange("p t f -> p (t f)")
    for r in range(T * L):
        nc.gpsimd.indirect_dma_start(
            out=oflat[:, r * F : (r + 1) * F],
            out_offset=None,
            in_=tables_ap,
            in_offset=bass.IndirectOffsetOnAxis(ap=idx_all[:, r : r + 1], axis=0),
        )

    # ---- write out
    nc.sync.dma_start(out=out_dram[ti], in_=otile[:].rearrange("p t f -> p (t f)"))


def build_program(size_val=1.0):
    nc = bacc.Bacc("TRN2", target_bir_lowering=False, debug=False,
                   num_devices=N_CORES)
    x_t = nc.dram_tensor("x", [NTILES, P, 3 * T], f32, kind="ExternalInput")
    tables_t = nc.dram_tensor("tables", [TOTAL_PARAMS, F], f32, kind="ExternalInput")
    out_t = nc.dram_tensor("out", [NTILES, P, T * L * F], f32, kind="ExternalOutput")

    x_ap = x_t.ap()
    tables_ap = tables_t.ap()
    out_ap = out_t.ap()

    with tile.TileContext(nc) as tc:
        with (
            tc.tile_pool(name="io", bufs=2) as io,
            tc.tile_pool(name="work", bufs=2) as work,
            tc.tile_pool(name="idxp", bufs=2) as idxp,
            tc.tile_pool(name="outp", bufs=2) as outp,
        ):
            for ti in range(NTILES):
                _emit_tile_program(
                    nc, tc, (io, work, idxp, outp), x_ap, tables_ap, out_ap, ti,
                    size_val,
                )
    nc.compile()
    return nc


_CACHE = {}


def _get_program(size_val):
    key = float(size_val)
    if key not in _CACHE:
        _CACHE[key] = build_program(key)
    return _CACHE[key]


def run(inputs, tables, size, trace=False):
    size_val = float(np.asarray(size))
    nc = _get_program(size_val)

    x = np.ascontiguousarray(np.asarray(inputs, dtype=np.float32))
    tb = np.ascontiguousarray(np.asarray(tables, dtype=np.float32))
    assert x.shape == (B, 3) and tb.shape == (TOTAL_PARAMS, F)

    in_maps = []
    for i in range(N_CORES):
        xs = x[i * B_CORE : (i + 1) * B_CORE].reshape(NTILES, P, 3 * T)
        in_maps.append({"x": xs, "tables": tb})

    res = run_bass_kernel_spmd(nc, in_maps, list(range(N_CORES)), trace=trace)
    outs = [
        res.results[i]["out"].reshape(B_CORE, L * F) for i in range(N_CORES)
    ]
    full = np.concatenate(outs, axis=0)
    return full, res


def kernel(inputs, tables, size):
    out, _ = run(inputs, tables, size, trace=False)
    return out
